# revision 22
# baseline (speedup 1.0000x reference)
"""Trainium2 Bass kernel for nn_BaselineDNN (ragged embedding-bag + MLP).

Per-core pipeline (8-way data parallel over the batch):
  - Host: fuse weights once: T1 = emb_table @ W1.T  [V, 128] (the masked
    mean commutes with the first linear layer), so the device gathers
    256B bf16 rows and skips the W1 matmul.
  - Host: globally sort batches by length desc + composition, deal to
    cores. Tokens are compacted class-major by table chunk (dma_gather
    indices are int16 -> 4 chunks of <=32768 rows) into GROUPS of up to
    8 tiles sized by the max-over-cores token count of a k-range, so all
    8 cores share ONE canonical instruction structure (SPMD) with only
    ~3.6% padding (id-0 rows, masked out). Canonical per-tile matmul
    windows are the union of the cores' batch spans. Groups are emitted
    in reverse k order (short batches first) and the final group is
    tapered so the tail drain is short.
  - Device: dma_gather (rotating over 4 SWDGE queues) fetches projected
    bf16 rows; each [128tok x 128h] tile feeds the PE as the stationary
    operand against a host-built bf16 mask (carrying 1/len) so PSUM
    accumulates (W1 @ rep).T; then relu(+b1) on DVE -> W2 (bf16) ->
    sigmoid (+b2, act table preloaded at kernel start).
"""

import os
from contextlib import ExitStack

import numpy as np
from ml_dtypes import bfloat16

import concourse.bass as bass
import concourse.bacc as bacc
import concourse.mybir as mybir
import concourse.tile as tile
from concourse._compat import get_trn_type
from concourse.bass_utils import run_bass_kernel_spmd

NCORES = 8
P = 128            # partitions
GTILES = 8         # max gather tiles per dma_gather (65 desc/lane packet cap)
BANKC = 256        # psum accumulator columns per bank tile (f32)
CHUNK = 32768      # table rows per gather chunk (int16 index limit)
NQ = 4             # SWDGE queues for gather descriptor generation

LAST_RESULT = None  # BassKernelResults of the most recent run (for test.py)

_NC_CACHE = {}


def _build_structure(counts, V):
    """Canonical structure from per-(row k, core, chunk) counts.

    counts: [Bc, NCORES, NCH]. Groups are k-ranges per chunk class sized so
    the canonical (max-over-cores) token count fits GTILES*P slots. Within a
    group each core packs real tokens first; only the canonical tile count
    and per-tile batch windows are shared instruction structure. Groups are
    emitted in reverse k order (short batches first) so the final group's
    windows are narrow, and the very last group is tapered."""
    Bc, W, NCH = counts.shape

    raw = []   # (c, k0, k1, gl, tiles[(w0,w1)])
    for c in range(NCH):
        cum = np.zeros((Bc + 1, W), np.int64)
        cum[1:] = np.cumsum(counts[:, :, c], axis=0)
        k = 0
        while k < Bc:
            lo, hi = k + 1, Bc
            while lo < hi:
                mid = (lo + hi + 1) // 2
                if int((cum[mid] - cum[k]).max()) <= GTILES * P:
                    lo = mid
                else:
                    hi = mid - 1
            k2 = lo
            G = int((cum[k2] - cum[k]).max())
            gl = max(1, (G + P - 1) // P)
            ccum = cum[k:k2 + 1] - cum[k]
            tiles = []
            for j in range(gl):
                s0, s1 = j * P, (j + 1) * P - 1
                w0, w1 = Bc, -1
                for core in range(W):
                    n = int(ccum[-1, core])
                    if s0 >= n:
                        continue
                    e = min(s1, n - 1)
                    kf = int(np.searchsorted(ccum[:, core], s0, "right")) - 1
                    kl = int(np.searchsorted(ccum[:, core], e, "right")) - 1
                    w0 = min(w0, k + kf)
                    w1 = max(w1, k + kl)
                if w1 < w0:
                    w0, w1 = k, k
                tiles.append((w0, w1))
            raw.append((c, k, k2, gl, tiles))
            k = k2

    raw.reverse()   # short-batch (high k) groups first; low-k groups last

    # taper: split the final group into [gl-2, 1, 1] so the last gather's
    # DMA drain is short
    c, k0, k1, gl, tiles = raw[-1]
    pieces = [(c, k0, k1, gl, tiles, None)]
    if gl > 2:
        pieces = []
        s0 = 0
        for ngl in (gl - 2, 1, 1):
            pieces.append((c, k0, k1, ngl, tiles[s0:s0 + ngl],
                           (s0 * P, (s0 + ngl) * P)))
            s0 += ngl
    raw = [(cc, a, b, g, t, None) for (cc, a, b, g, t) in raw[:-1]] + pieces

    groups = []
    Tstart = 0
    col_off = 0
    for (c, k0, k1, gl, tiles, sub) in raw:
        g = dict(c=c, k0=k0, k1=k1, gl=gl, coff=col_off, Tstart=Tstart,
                 tiles=tiles)
        if sub is not None:
            g["sub"] = sub
        groups.append(g)
        col_off += gl * P // 16
        Tstart += gl

    T = Tstart
    widths = [w1 - w0 + 1 for g in groups for (w0, w1) in g["tiles"]]
    moff = np.zeros(T + 1, np.int64)
    moff[1:] = np.cumsum(widths)
    Wtot = int(moff[-1])

    nbank = (Bc + BANKC - 1) // BANKC
    flat = [(w0, w1) for g in groups for (w0, w1) in g["tiles"]]
    last_tile = {}
    for jg, (w0, w1) in enumerate(flat):
        for b in range(w0 // BANKC, w1 // BANKC + 1):
            last_tile[b] = jg
    parts = []  # per global tile: list of (bank, col0, col1, mask_local_off, stop)
    for jg, (w0, w1) in enumerate(flat):
        pj = []
        for b in range(w0 // BANKC, w1 // BANKC + 1):
            kb0 = max(w0, b * BANKC)
            kb1 = min(w1, b * BANKC + BANKC - 1)
            pj.append((b, kb0 - b * BANKC, kb1 - b * BANKC + 1,
                       kb0 - w0, jg == last_tile[b]))
        parts.append(pj)

    return dict(Bc=Bc, NCH=NCH, groups=groups, T=T, moff=moff, Wtot=Wtot,
                nbank=nbank, parts=parts, idx_cols=col_off)


def _trace_nc(st, V, DP):
    """Build + compile the SPMD Bacc program; DP = projected dim (128)."""
    Bc, Wtot = st["Bc"], st["Wtot"]
    moff, parts = st["moff"], st["parts"]
    nbank = st["nbank"]
    groups = st["groups"]
    idx_cols = st["idx_cols"]
    f32 = mybir.dt.float32
    bf16 = mybir.dt.bfloat16
    assert DP == P

    nc = bacc.Bacc(
        get_trn_type() or "TRN2",
        target_bir_lowering=False,
        debug=False,
        num_devices=NCORES,
        num_swdge_queues=NQ,
        dynamic_dma_scratch_size=32768,
    )
    ngroups = len(groups)
    t1_d = nc.dram_tensor("t1", [V, DP], bf16, kind="ExternalInput")
    idx_d = nc.dram_tensor("idx", [P, idx_cols], mybir.dt.int16,
                           kind="ExternalInput")
    mask_d = nc.dram_tensor("mask", [P, Wtot], bf16, kind="ExternalInput")
    b1_d = nc.dram_tensor("b1c", [P, 1], f32, kind="ExternalInput")
    w2t_d = nc.dram_tensor("w2t", [P, 1], bf16, kind="ExternalInput")
    b2_d = nc.dram_tensor("b2c", [1, 1], f32, kind="ExternalInput")
    y_d = nc.dram_tensor("y", [1, Bc], f32, kind="ExternalOutput")

    with tile.TileContext(nc) as tc, ExitStack() as ctx:
        consts = ctx.enter_context(tc.tile_pool(name="consts", bufs=1))
        gpool = ctx.enter_context(tc.tile_pool(name="gather", bufs=8))
        psum = ctx.enter_context(tc.tile_pool(name="psum", bufs=1, space="PSUM"))
        sb = ctx.enter_context(tc.tile_pool(name="sb", bufs=1))

        # Interleave idx chunks and mask slices on ONE queue: in-order
        # completion gives the first gather its idx fast while mask slices
        # land just ahead of the matmuls that need them.
        idx_sb = consts.tile([P, idx_cols], mybir.dt.int16)
        mask_sb = consts.tile([P, Wtot], bf16)
        bounds = [g["coff"] for g in groups] + [idx_cols]
        chunks = [(bounds[0], bounds[1])]
        i = 1
        while i < len(groups):
            j = min(i + 4, len(groups))
            chunks.append((bounds[i], bounds[j]))
            i = j
        nmsk = 8
        msk = [(Wtot * i // nmsk, Wtot * (i + 1) // nmsk) for i in range(nmsk)]
        mi = 0
        for ci, (lo, hi) in enumerate(chunks):
            if hi > lo:
                eng = nc.scalar if ci < 2 else nc.sync
                eng.dma_start(out=idx_sb[:, lo:hi],
                              in_=idx_d.ap()[:, lo:hi])
            if ci % 2 == 0 and mi < nmsk:
                mlo, mhi = msk[mi]
                if mhi > mlo:
                    nc.sync.dma_start(out=mask_sb[:, mlo:mhi],
                                      in_=mask_d.ap()[:, mlo:mhi])
                mi += 1
        while mi < nmsk:
            mlo, mhi = msk[mi]
            if mhi > mlo:
                nc.sync.dma_start(out=mask_sb[:, mlo:mhi],
                                  in_=mask_d.ap()[:, mlo:mhi])
            mi += 1
        b1_sb = consts.tile([P, 1], f32)
        nc.sync.dma_start(out=b1_sb[:], in_=b1_d.ap())
        w2t_sb = consts.tile([P, 1], bf16)
        nc.sync.dma_start(out=w2t_sb[:], in_=w2t_d.ap())
        b2_sb = consts.tile([1, 1], f32)
        nc.sync.dma_start(out=b2_sb[:], in_=b2_d.ap())

        # Preload the sigmoid activation table off the critical tail.
        sig_warm = sb.tile([1, 1], f32)
        nc.scalar.activation(
            sig_warm[:], b2_sb[0:1, 0:1],
            mybir.ActivationFunctionType.Sigmoid,
        )

        # rep_ps[b] accumulates (W1 @ rep).T : [128 h, BANKC batches]
        rep_ps = [psum.tile([P, BANKC], f32, tag=f"rep{b}", name=f"rep{b}")
                  for b in range(nbank)]
        # Open each PSUM accumulation group with a full-bank zeroing matmul
        # (K=1, bf16) so every staircase matmul is a pure accumulate.
        zrow = consts.tile([1, BANKC], mybir.dt.bfloat16)
        nc.vector.memset(zrow, 0)
        for b in range(nbank):
            nc.tensor.matmul(
                rep_ps[b][:], zrow[0:1, 0:P], zrow[0:1, :],
                start=True, stop=False,
            )

        for gi, g in enumerate(groups):
            c, gl, coff, Tstart = g["c"], g["gl"], g["coff"], g["Tstart"]
            rows = min(CHUNK, V - c * CHUNK)
            chunk_ap = t1_d.ap()[c * CHUNK: c * CHUNK + rows, :]
            gt = gpool.tile([P, GTILES, DP], bf16, tag="gt")
            nc.gpsimd.dma_gather(
                gt[:, :gl, :],
                chunk_ap,
                idx_sb[:, coff: coff + gl * P // 16],
                gl * P,
                gl * P,
                DP,
                single_packet=False,
                queue_num=gi % NQ,
            )
            for jl in range(gl):
                jg = Tstart + jl
                mo = int(moff[jg])
                lhsT = gt[:, jl, :]
                for (b, c0, c1, ml, sp_flag) in parts[jg]:
                    nc.tensor.matmul(
                        rep_ps[b][:, c0:c1],
                        lhsT,
                        mask_sb[:, mo + ml: mo + ml + (c1 - c0)],
                        start=False,
                        stop=sp_flag,
                    )

        # ---- tail: h = relu(rep_proj + b1) on DVE (bf16 out);
        #      y = sigmoid(W2 @ h + b2) ----
        h_sb = sb.tile([P, Bc], bf16)
        for b in range(nbank):
            nc.vector.tensor_scalar(
                h_sb[:, b * BANKC:(b + 1) * BANKC],
                rep_ps[b][:],
                b1_sb[:, 0:1],
                0.0,
                mybir.AluOpType.add,
                mybir.AluOpType.max,
            )
        l_ps = [psum.tile([1, BANKC], f32, tag=f"lps{b}", name=f"lps{b}")
                for b in range(nbank)]
        y_sb = sb.tile([1, Bc], f32)
        for b in range(nbank):
            nc.tensor.matmul(
                l_ps[b][:],
                w2t_sb[:],
                h_sb[:, b * BANKC:(b + 1) * BANKC],
                start=True, stop=True,
            )
            nc.scalar.activation(
                y_sb[:, b * BANKC:(b + 1) * BANKC],
                l_ps[b][:],
                mybir.ActivationFunctionType.Sigmoid,
                bias=b2_sb[0:1, 0:1],
            )
        nc.sync.dma_start(out=y_d.ap(), in_=y_sb[:])

    nc.compile()
    return nc


def _local_search(perm, nb, iters=4):
    """Swap batches between adjacent rows to tighten per-row max counts."""
    Bc, W = perm.shape
    for _ in range(iters):
        changed = False
        for parity in (0, 1):
            ks = np.arange(parity, Bc - 1, 2)
            for a in range(W):
                for b in range(W):
                    Qa = nb[perm[ks]]
                    Qb = nb[perm[ks + 1]]
                    old = Qa.max(1).sum(1) + Qb.max(1).sum(1)
                    Qa2 = Qa.copy()
                    Qb2 = Qb.copy()
                    Qa2[:, a] = nb[perm[ks + 1, b]]
                    Qb2[:, b] = nb[perm[ks, a]]
                    new = Qa2.max(1).sum(1) + Qb2.max(1).sum(1)
                    win = new < old
                    if win.any():
                        kw = ks[win]
                        tmp = perm[kw, a].copy()
                        perm[kw, a] = perm[kw + 1, b]
                        perm[kw + 1, b] = tmp
                        changed = True
        if not changed:
            break
    return perm


def _prepare(x, lengths, emb_table, W1, b1, W2, b2):
    """Host-side sharding: weight fusion + canonical structure + arrays."""
    x = np.asarray(x)
    lengths = np.asarray(lengths).astype(np.int64)
    B, L = x.shape
    V, D = emb_table.shape
    Bc = B // NCORES
    NCH = (V + CHUNK - 1) // CHUNK

    # weight fusion: masked-mean commutes with W1
    W1f = np.asarray(W1, np.float32)
    t1 = np.ascontiguousarray(
        np.asarray(emb_table, np.float32) @ W1f.T).astype(bfloat16)  # [V, 128]
    DP = t1.shape[1]

    lpos = np.arange(L, dtype=np.int64)
    chunk_of = (x >> 15).astype(np.int64)     # CHUNK == 1 << 15
    valid_all = lpos[None, :] < lengths[:, None]
    nb = np.zeros((B, NCH), np.int64)         # per-batch per-chunk counts
    for c in range(NCH):
        nb[:, c] = ((chunk_of == c) & valid_all).sum(axis=1)

    # Sort by length desc, cluster chunk compositions within near-equal
    # length bands, then local-search swaps between adjacent rows.
    order = np.argsort(-lengths, kind="stable")
    ln_sorted = lengths[order]
    BAND = 8
    i = 0
    while i < B:
        j = i
        while j < B and ln_sorted[i] - ln_sorted[j] < BAND:
            j += 1
        band = order[i:j]
        v = nb[band]
        order[i:j] = band[np.lexsort(
            (v[:, 3 % NCH], v[:, 2 % NCH], v[:, 1 % NCH], v[:, 0]))]
        i = j
    perm = order.reshape(Bc, NCORES)          # [k, core] -> original batch idx
    perm = _local_search(perm, nb)
    plen = lengths[perm]                      # [k, core] actual lengths

    counts = nb[perm]                         # [Bc, NCORES, NCH]
    st = _build_structure(counts, V)
    groups, moff = st["groups"], st["moff"]
    Wtot, idx_cols = st["Wtot"], st["idx_cols"]

    inv_len = (1.0 / plen.astype(np.float64)).astype(np.float32)

    idx_cores = []
    mask_cores = []
    for core in range(NCORES):
        xl = x[perm[:, core]]
        validc = lpos[None, :] < plen[:, core][:, None]
        idx16 = np.zeros((P, idx_cols), np.int16)
        mask_host = np.zeros((P, Wtot), np.float32)
        for c in range(NCH):
            sel_mask = validc & (chunk_of[perm[:, core]] == c)
            sel = np.nonzero(sel_mask.ravel())[0]
            k_sel = sel // L                       # sorted ascending
            ids = (xl.ravel()[sel] & (CHUNK - 1)).astype(np.int16)
            cnt = np.bincount(k_sel, minlength=Bc)
            ccum = np.zeros(Bc + 1, np.int64)
            ccum[1:] = np.cumsum(cnt)
            for gi, g in enumerate(groups):
                if g["c"] != c:
                    continue
                k0, k1, gl, coff, Tstart = (g["k0"], g["k1"], g["gl"],
                                            g["coff"], g["Tstart"])
                a, bnd = int(ccum[k0]), int(ccum[k1])
                if "sub" in g:                     # tapered sub-group
                    lo, hi = g["sub"]
                    a, bnd = a + lo, min(bnd, a + hi)
                n = max(0, bnd - a)
                gslots = gl * P
                seg = np.zeros(gslots, np.int16)
                seg[:n] = ids[a:bnd]
                # packed slot s -> batch row k_sel[a+s], tile j = s//P
                if n:
                    s = np.arange(n)
                    kk = k_sel[a:bnd]             # global row index
                    j_in_g = s // P
                    jg = Tstart + j_in_g
                    w0 = np.array([g["tiles"][j][0] for j in range(gl)])
                    col = moff[jg] + (kk - w0[j_in_g])
                    mask_host[s % P, col] = inv_len[kk, core]
                wrap = seg.reshape(-1, 16).T
                for st8 in range(P // 16):
                    idx16[16 * st8:16 * st8 + 16,
                          coff: coff + gslots // 16] = wrap
        idx_cores.append(idx16)
        mask_cores.append(mask_host.astype(bfloat16))

    b1c = np.asarray(b1, np.float32).reshape(P, 1)
    w2t = np.ascontiguousarray(
        np.asarray(W2, np.float32).reshape(1, P).T).astype(bfloat16)
    b2c = np.asarray(b2, np.float32).reshape(1, 1)

    in_maps = []
    for core in range(NCORES):
        in_maps.append({
            "t1": t1,
            "idx": idx_cores[core],
            "mask": mask_cores[core],
            "b1c": b1c,
            "w2t": w2t,
            "b2c": b2c,
        })
    return st, perm, in_maps, (V, DP)


def kernel(x, lengths, emb_table, W1, b1, W2, b2):
    global LAST_RESULT
    st, perm, in_maps, (V, DP) = _prepare(x, lengths, emb_table, W1, b1, W2, b2)

    key = (st["T"], st["Wtot"], V, DP, st["Bc"])
    nc = _NC_CACHE.get(key)
    if nc is None:
        nc = _trace_nc(st, V, DP)
        _NC_CACHE[key] = nc

    trace = bool(int(os.environ.get("KERNEL_TRACE", "0")))
    res = run_bass_kernel_spmd(nc, in_maps, core_ids=list(range(NCORES)),
                               trace=trace)
    LAST_RESULT = res

    B = perm.size
    out = np.zeros(B, np.float32)
    for c in range(NCORES):
        out[perm[:, c]] = res.results[c]["y"][0]
    return out


# revision 23
# speedup vs baseline: 1.0068x; 1.0068x over previous
"""Trainium2 Bass kernel for nn_BaselineDNN (ragged embedding-bag + MLP).

Per-core pipeline (8-way data parallel over the batch):
  - Host: fuse weights once: T1 = emb_table @ W1.T  [V, 128] (the masked
    mean commutes with the first linear layer), so the device gathers
    256B bf16 rows and skips the W1 matmul.
  - Host: globally sort batches by length desc + composition, deal to
    cores. Tokens are compacted class-major by table chunk (dma_gather
    indices are int16 -> 4 chunks of <=32768 rows) into GROUPS of up to
    8 tiles sized by the max-over-cores token count of a k-range, so all
    8 cores share ONE canonical instruction structure (SPMD) with only
    ~3.6% padding (id-0 rows, masked out). Canonical per-tile matmul
    windows are the union of the cores' batch spans. Groups are emitted
    in reverse k order (short batches first) and the final group is
    tapered so the tail drain is short.
  - Device: dma_gather (rotating over 4 SWDGE queues) fetches projected
    bf16 rows; each [128tok x 128h] tile feeds the PE as the stationary
    operand against a host-built bf16 mask (carrying 1/len) so PSUM
    accumulates (W1 @ rep).T; then relu(+b1) on DVE -> W2 (bf16) ->
    sigmoid (+b2, act table preloaded at kernel start).
"""

import os
from contextlib import ExitStack

import numpy as np
from ml_dtypes import bfloat16

import concourse.bass as bass
import concourse.bacc as bacc
import concourse.mybir as mybir
import concourse.tile as tile
from concourse._compat import get_trn_type
from concourse.bass_utils import run_bass_kernel_spmd

NCORES = 8
P = 128            # partitions
GTILES = 8         # max gather tiles per dma_gather (65 desc/lane packet cap)
BANKC = 256        # psum accumulator columns per bank tile (f32)
CHUNK = 32768      # table rows per gather chunk (int16 index limit)
NQ = 4             # SWDGE queues for gather descriptor generation

LAST_RESULT = None  # BassKernelResults of the most recent run (for test.py)

_NC_CACHE = {}


def _build_structure(counts, V):
    """Canonical structure from per-(row k, core, chunk) counts.

    counts: [Bc, NCORES, NCH]. Groups are k-ranges per chunk class sized so
    the canonical (max-over-cores) token count fits GTILES*P slots. Within a
    group each core packs real tokens first; only the canonical tile count
    and per-tile batch windows are shared instruction structure. Groups are
    emitted in reverse k order (short batches first) so the final group's
    windows are narrow, and the very last group is tapered."""
    Bc, W, NCH = counts.shape

    raw = []   # (c, k0, k1, gl, tiles[(w0,w1)])
    for c in range(NCH):
        cum = np.zeros((Bc + 1, W), np.int64)
        cum[1:] = np.cumsum(counts[:, :, c], axis=0)
        k = 0
        while k < Bc:
            lo, hi = k + 1, Bc
            while lo < hi:
                mid = (lo + hi + 1) // 2
                if int((cum[mid] - cum[k]).max()) <= GTILES * P:
                    lo = mid
                else:
                    hi = mid - 1
            k2 = lo
            G = int((cum[k2] - cum[k]).max())
            gl = max(1, (G + P - 1) // P)
            ccum = cum[k:k2 + 1] - cum[k]
            tiles = []
            for j in range(gl):
                s0, s1 = j * P, (j + 1) * P - 1
                w0, w1 = Bc, -1
                for core in range(W):
                    n = int(ccum[-1, core])
                    if s0 >= n:
                        continue
                    e = min(s1, n - 1)
                    kf = int(np.searchsorted(ccum[:, core], s0, "right")) - 1
                    kl = int(np.searchsorted(ccum[:, core], e, "right")) - 1
                    w0 = min(w0, k + kf)
                    w1 = max(w1, k + kl)
                if w1 < w0:
                    w0, w1 = k, k
                tiles.append((w0, w1))
            raw.append((c, k, k2, gl, tiles))
            k = k2

    raw.reverse()   # short-batch (high k) groups first; low-k groups last

    # taper: split the final group into [gl-2, 1, 1] so the last gather's
    # DMA drain is short
    c, k0, k1, gl, tiles = raw[-1]
    pieces = [(c, k0, k1, gl, tiles, None)]
    if gl > 2:
        pieces = []
        s0 = 0
        for ngl in (gl - 2, 1, 1):
            pieces.append((c, k0, k1, ngl, tiles[s0:s0 + ngl],
                           (s0 * P, (s0 + ngl) * P)))
            s0 += ngl
    raw = [(cc, a, b, g, t, None) for (cc, a, b, g, t) in raw[:-1]] + pieces

    groups = []
    Tstart = 0
    col_off = 0
    for (c, k0, k1, gl, tiles, sub) in raw:
        g = dict(c=c, k0=k0, k1=k1, gl=gl, coff=col_off, Tstart=Tstart,
                 tiles=tiles)
        if sub is not None:
            g["sub"] = sub
        groups.append(g)
        col_off += gl * P // 16
        Tstart += gl

    T = Tstart
    widths = [w1 - w0 + 1 for g in groups for (w0, w1) in g["tiles"]]
    moff = np.zeros(T + 1, np.int64)
    moff[1:] = np.cumsum(widths)
    Wtot = int(moff[-1])

    nbank = (Bc + BANKC - 1) // BANKC
    flat = [(w0, w1) for g in groups for (w0, w1) in g["tiles"]]
    last_tile = {}
    for jg, (w0, w1) in enumerate(flat):
        for b in range(w0 // BANKC, w1 // BANKC + 1):
            last_tile[b] = jg
    parts = []  # per global tile: list of (bank, col0, col1, mask_local_off, stop)
    for jg, (w0, w1) in enumerate(flat):
        pj = []
        for b in range(w0 // BANKC, w1 // BANKC + 1):
            kb0 = max(w0, b * BANKC)
            kb1 = min(w1, b * BANKC + BANKC - 1)
            pj.append((b, kb0 - b * BANKC, kb1 - b * BANKC + 1,
                       kb0 - w0, jg == last_tile[b]))
        parts.append(pj)

    return dict(Bc=Bc, NCH=NCH, groups=groups, T=T, moff=moff, Wtot=Wtot,
                nbank=nbank, parts=parts, idx_cols=col_off)


def _trace_nc(st, V, DP):
    """Build + compile the SPMD Bacc program; DP = projected dim (128)."""
    Bc, Wtot = st["Bc"], st["Wtot"]
    moff, parts = st["moff"], st["parts"]
    nbank = st["nbank"]
    groups = st["groups"]
    idx_cols = st["idx_cols"]
    f32 = mybir.dt.float32
    bf16 = mybir.dt.bfloat16
    assert DP == P

    nc = bacc.Bacc(
        get_trn_type() or "TRN2",
        target_bir_lowering=False,
        debug=False,
        num_devices=NCORES,
        num_swdge_queues=NQ,
        dynamic_dma_scratch_size=32768,
    )
    ngroups = len(groups)
    t1_d = nc.dram_tensor("t1", [V, DP], bf16, kind="ExternalInput")
    idx_d = nc.dram_tensor("idx", [P, idx_cols], mybir.dt.int16,
                           kind="ExternalInput")
    mask_d = nc.dram_tensor("mask", [P, Wtot], bf16, kind="ExternalInput")
    b1_d = nc.dram_tensor("b1c", [P, 1], f32, kind="ExternalInput")
    w2t_d = nc.dram_tensor("w2t", [P, 1], bf16, kind="ExternalInput")
    b2_d = nc.dram_tensor("b2c", [1, 1], f32, kind="ExternalInput")
    y_d = nc.dram_tensor("y", [1, Bc], f32, kind="ExternalOutput")

    with tile.TileContext(nc) as tc, ExitStack() as ctx:
        consts = ctx.enter_context(tc.tile_pool(name="consts", bufs=1))
        gpool = ctx.enter_context(tc.tile_pool(name="gather", bufs=12))
        psum = ctx.enter_context(tc.tile_pool(name="psum", bufs=1, space="PSUM"))
        sb = ctx.enter_context(tc.tile_pool(name="sb", bufs=1))

        # Interleave idx chunks and mask slices on ONE queue: in-order
        # completion gives the first gather its idx fast while mask slices
        # land just ahead of the matmuls that need them.
        idx_sb = consts.tile([P, idx_cols], mybir.dt.int16)
        mask_sb = consts.tile([P, Wtot], bf16)
        bounds = [g["coff"] for g in groups] + [idx_cols]
        chunks = [(bounds[0], bounds[1])]
        i = 1
        while i < len(groups):
            j = min(i + 4, len(groups))
            chunks.append((bounds[i], bounds[j]))
            i = j
        nmsk = 8
        msk = [(Wtot * i // nmsk, Wtot * (i + 1) // nmsk) for i in range(nmsk)]
        mi = 0
        for ci, (lo, hi) in enumerate(chunks):
            if hi > lo:
                eng = nc.scalar if ci < 2 else nc.sync
                eng.dma_start(out=idx_sb[:, lo:hi],
                              in_=idx_d.ap()[:, lo:hi])
            if ci % 2 == 0 and mi < nmsk:
                mlo, mhi = msk[mi]
                if mhi > mlo:
                    nc.sync.dma_start(out=mask_sb[:, mlo:mhi],
                                      in_=mask_d.ap()[:, mlo:mhi])
                mi += 1
        while mi < nmsk:
            mlo, mhi = msk[mi]
            if mhi > mlo:
                nc.sync.dma_start(out=mask_sb[:, mlo:mhi],
                                  in_=mask_d.ap()[:, mlo:mhi])
            mi += 1
        b1_sb = consts.tile([P, 1], f32)
        nc.sync.dma_start(out=b1_sb[:], in_=b1_d.ap())
        w2t_sb = consts.tile([P, 1], bf16)
        nc.sync.dma_start(out=w2t_sb[:], in_=w2t_d.ap())
        b2_sb = consts.tile([1, 1], f32)
        nc.sync.dma_start(out=b2_sb[:], in_=b2_d.ap())

        # Preload the sigmoid activation table off the critical tail.
        sig_warm = sb.tile([1, 1], f32)
        nc.scalar.activation(
            sig_warm[:], b2_sb[0:1, 0:1],
            mybir.ActivationFunctionType.Sigmoid,
        )

        # rep_ps[b] accumulates (W1 @ rep).T : [128 h, BANKC batches]
        rep_ps = [psum.tile([P, BANKC], f32, tag=f"rep{b}", name=f"rep{b}")
                  for b in range(nbank)]
        # Open each PSUM accumulation group with a full-bank zeroing matmul
        # (K=1, bf16) so every staircase matmul is a pure accumulate.
        zrow = consts.tile([1, BANKC], mybir.dt.bfloat16)
        nc.vector.memset(zrow, 0)
        for b in range(nbank):
            nc.tensor.matmul(
                rep_ps[b][:], zrow[0:1, 0:P], zrow[0:1, :],
                start=True, stop=False,
            )

        for gi, g in enumerate(groups):
            c, gl, coff, Tstart = g["c"], g["gl"], g["coff"], g["Tstart"]
            rows = min(CHUNK, V - c * CHUNK)
            chunk_ap = t1_d.ap()[c * CHUNK: c * CHUNK + rows, :]
            gt = gpool.tile([P, GTILES, DP], bf16, tag="gt")
            nc.gpsimd.dma_gather(
                gt[:, :gl, :],
                chunk_ap,
                idx_sb[:, coff: coff + gl * P // 16],
                gl * P,
                gl * P,
                DP,
                single_packet=False,
                queue_num=gi % NQ,
            )
            for jl in range(gl):
                jg = Tstart + jl
                mo = int(moff[jg])
                lhsT = gt[:, jl, :]
                for (b, c0, c1, ml, sp_flag) in parts[jg]:
                    nc.tensor.matmul(
                        rep_ps[b][:, c0:c1],
                        lhsT,
                        mask_sb[:, mo + ml: mo + ml + (c1 - c0)],
                        start=False,
                        stop=sp_flag,
                    )

        # ---- tail: h = relu(rep_proj + b1) on DVE (bf16 out);
        #      y = sigmoid(W2 @ h + b2) ----
        h_sb = sb.tile([P, Bc], bf16)
        for b in range(nbank):
            nc.vector.tensor_scalar(
                h_sb[:, b * BANKC:(b + 1) * BANKC],
                rep_ps[b][:],
                b1_sb[:, 0:1],
                0.0,
                mybir.AluOpType.add,
                mybir.AluOpType.max,
            )
        l_ps = [psum.tile([1, BANKC], f32, tag=f"lps{b}", name=f"lps{b}")
                for b in range(nbank)]
        y_sb = sb.tile([1, Bc], f32)
        for b in range(nbank):
            nc.tensor.matmul(
                l_ps[b][:],
                w2t_sb[:],
                h_sb[:, b * BANKC:(b + 1) * BANKC],
                start=True, stop=True,
            )
            nc.scalar.activation(
                y_sb[:, b * BANKC:(b + 1) * BANKC],
                l_ps[b][:],
                mybir.ActivationFunctionType.Sigmoid,
                bias=b2_sb[0:1, 0:1],
            )
        nc.sync.dma_start(out=y_d.ap(), in_=y_sb[:])

    nc.compile()
    return nc


def _local_search(perm, nb, iters=4):
    """Swap batches between adjacent rows to tighten per-row max counts."""
    Bc, W = perm.shape
    for _ in range(iters):
        changed = False
        for parity in (0, 1):
            ks = np.arange(parity, Bc - 1, 2)
            for a in range(W):
                for b in range(W):
                    Qa = nb[perm[ks]]
                    Qb = nb[perm[ks + 1]]
                    old = Qa.max(1).sum(1) + Qb.max(1).sum(1)
                    Qa2 = Qa.copy()
                    Qb2 = Qb.copy()
                    Qa2[:, a] = nb[perm[ks + 1, b]]
                    Qb2[:, b] = nb[perm[ks, a]]
                    new = Qa2.max(1).sum(1) + Qb2.max(1).sum(1)
                    win = new < old
                    if win.any():
                        kw = ks[win]
                        tmp = perm[kw, a].copy()
                        perm[kw, a] = perm[kw + 1, b]
                        perm[kw + 1, b] = tmp
                        changed = True
        if not changed:
            break
    return perm


def _prepare(x, lengths, emb_table, W1, b1, W2, b2):
    """Host-side sharding: weight fusion + canonical structure + arrays."""
    x = np.asarray(x)
    lengths = np.asarray(lengths).astype(np.int64)
    B, L = x.shape
    V, D = emb_table.shape
    Bc = B // NCORES
    NCH = (V + CHUNK - 1) // CHUNK

    # weight fusion: masked-mean commutes with W1
    W1f = np.asarray(W1, np.float32)
    t1 = np.ascontiguousarray(
        np.asarray(emb_table, np.float32) @ W1f.T).astype(bfloat16)  # [V, 128]
    DP = t1.shape[1]

    lpos = np.arange(L, dtype=np.int64)
    chunk_of = (x >> 15).astype(np.int64)     # CHUNK == 1 << 15
    valid_all = lpos[None, :] < lengths[:, None]
    nb = np.zeros((B, NCH), np.int64)         # per-batch per-chunk counts
    for c in range(NCH):
        nb[:, c] = ((chunk_of == c) & valid_all).sum(axis=1)

    # Sort by length desc, cluster chunk compositions within near-equal
    # length bands, then local-search swaps between adjacent rows.
    order = np.argsort(-lengths, kind="stable")
    ln_sorted = lengths[order]
    BAND = 8
    i = 0
    while i < B:
        j = i
        while j < B and ln_sorted[i] - ln_sorted[j] < BAND:
            j += 1
        band = order[i:j]
        v = nb[band]
        order[i:j] = band[np.lexsort(
            (v[:, 3 % NCH], v[:, 2 % NCH], v[:, 1 % NCH], v[:, 0]))]
        i = j
    perm = order.reshape(Bc, NCORES)          # [k, core] -> original batch idx
    perm = _local_search(perm, nb)
    plen = lengths[perm]                      # [k, core] actual lengths

    counts = nb[perm]                         # [Bc, NCORES, NCH]
    st = _build_structure(counts, V)
    groups, moff = st["groups"], st["moff"]
    Wtot, idx_cols = st["Wtot"], st["idx_cols"]

    inv_len = (1.0 / plen.astype(np.float64)).astype(np.float32)

    idx_cores = []
    mask_cores = []
    for core in range(NCORES):
        xl = x[perm[:, core]]
        validc = lpos[None, :] < plen[:, core][:, None]
        idx16 = np.zeros((P, idx_cols), np.int16)
        mask_host = np.zeros((P, Wtot), np.float32)
        for c in range(NCH):
            sel_mask = validc & (chunk_of[perm[:, core]] == c)
            sel = np.nonzero(sel_mask.ravel())[0]
            k_sel = sel // L                       # sorted ascending
            ids = (xl.ravel()[sel] & (CHUNK - 1)).astype(np.int16)
            cnt = np.bincount(k_sel, minlength=Bc)
            ccum = np.zeros(Bc + 1, np.int64)
            ccum[1:] = np.cumsum(cnt)
            for gi, g in enumerate(groups):
                if g["c"] != c:
                    continue
                k0, k1, gl, coff, Tstart = (g["k0"], g["k1"], g["gl"],
                                            g["coff"], g["Tstart"])
                a, bnd = int(ccum[k0]), int(ccum[k1])
                if "sub" in g:                     # tapered sub-group
                    lo, hi = g["sub"]
                    a, bnd = a + lo, min(bnd, a + hi)
                n = max(0, bnd - a)
                gslots = gl * P
                seg = np.zeros(gslots, np.int16)
                seg[:n] = ids[a:bnd]
                # packed slot s -> batch row k_sel[a+s], tile j = s//P
                if n:
                    s = np.arange(n)
                    kk = k_sel[a:bnd]             # global row index
                    j_in_g = s // P
                    jg = Tstart + j_in_g
                    w0 = np.array([g["tiles"][j][0] for j in range(gl)])
                    col = moff[jg] + (kk - w0[j_in_g])
                    mask_host[s % P, col] = inv_len[kk, core]
                wrap = seg.reshape(-1, 16).T
                for st8 in range(P // 16):
                    idx16[16 * st8:16 * st8 + 16,
                          coff: coff + gslots // 16] = wrap
        idx_cores.append(idx16)
        mask_cores.append(mask_host.astype(bfloat16))

    b1c = np.asarray(b1, np.float32).reshape(P, 1)
    w2t = np.ascontiguousarray(
        np.asarray(W2, np.float32).reshape(1, P).T).astype(bfloat16)
    b2c = np.asarray(b2, np.float32).reshape(1, 1)

    in_maps = []
    for core in range(NCORES):
        in_maps.append({
            "t1": t1,
            "idx": idx_cores[core],
            "mask": mask_cores[core],
            "b1c": b1c,
            "w2t": w2t,
            "b2c": b2c,
        })
    return st, perm, in_maps, (V, DP)


def kernel(x, lengths, emb_table, W1, b1, W2, b2):
    global LAST_RESULT
    st, perm, in_maps, (V, DP) = _prepare(x, lengths, emb_table, W1, b1, W2, b2)

    key = (st["T"], st["Wtot"], V, DP, st["Bc"])
    nc = _NC_CACHE.get(key)
    if nc is None:
        nc = _trace_nc(st, V, DP)
        _NC_CACHE[key] = nc

    trace = bool(int(os.environ.get("KERNEL_TRACE", "0")))
    res = run_bass_kernel_spmd(nc, in_maps, core_ids=list(range(NCORES)),
                               trace=trace)
    LAST_RESULT = res

    B = perm.size
    out = np.zeros(B, np.float32)
    for c in range(NCORES):
        out[perm[:, c]] = res.results[c]["y"][0]
    return out


# revision 25
# speedup vs baseline: 1.0366x; 1.0296x over previous
"""Trainium2 Bass kernel for nn_BaselineDNN (ragged embedding-bag + MLP).

Per-core pipeline (8-way data parallel over the batch):
  - Host: fuse weights once: T1 = emb_table @ W1.T  [V, 128] (the masked
    mean commutes with the first linear layer), so the device gathers
    256B bf16 rows and skips the W1 matmul.
  - Host: globally sort batches by length desc + composition, deal to
    cores. Tokens are compacted class-major by table chunk (dma_gather
    indices are int16 -> 4 chunks of <=32768 rows) into GROUPS of up to
    8 tiles sized by the max-over-cores token count of a k-range, so all
    8 cores share ONE canonical instruction structure (SPMD) with only
    ~3.6% padding (id-0 rows, masked out). Canonical per-tile matmul
    windows are the union of the cores' batch spans. Groups are emitted
    in reverse k order (short batches first) and the final group is
    tapered so the tail drain is short.
  - Device: dma_gather (rotating over 4 SWDGE queues) fetches projected
    bf16 rows; each [128tok x 128h] tile feeds the PE as the stationary
    operand against a host-built bf16 mask (carrying 1/len) so PSUM
    accumulates (W1 @ rep).T; then relu(+b1) on DVE -> W2 (bf16) ->
    sigmoid (+b2, act table preloaded at kernel start).
"""

import os
from contextlib import ExitStack

import numpy as np
from ml_dtypes import bfloat16

import concourse.bass as bass
import concourse.bacc as bacc
import concourse.mybir as mybir
import concourse.tile as tile
from concourse._compat import get_trn_type
from concourse.bass_utils import run_bass_kernel_spmd

NCORES = 8
P = 128            # partitions
GTILES = 8         # max gather tiles per dma_gather (65 desc/lane packet cap)
BANKC = 256        # psum accumulator columns per bank tile (f32)
CHUNK = 32768      # table rows per gather chunk (int16 index limit)
NQ = 4             # SWDGE queues for gather descriptor generation

LAST_RESULT = None  # BassKernelResults of the most recent run (for test.py)

_NC_CACHE = {}


def _build_structure(counts, V):
    """Canonical structure from per-(row k, core, chunk) counts.

    counts: [Bc, NCORES, NCH]. Groups are k-ranges per chunk class sized so
    the canonical (max-over-cores) token count fits GTILES*P slots. Within a
    group each core packs real tokens first; only the canonical tile count
    and per-tile batch windows are shared instruction structure. Groups are
    emitted in reverse k order (short batches first) so the final group's
    windows are narrow, and the very last group is tapered."""
    Bc, W, NCH = counts.shape

    raw = []   # (c, k0, k1, gl, tiles[(w0,w1)])
    for c in range(NCH):
        cum = np.zeros((Bc + 1, W), np.int64)
        cum[1:] = np.cumsum(counts[:, :, c], axis=0)
        k = 0
        while k < Bc:
            lo, hi = k + 1, Bc
            while lo < hi:
                mid = (lo + hi + 1) // 2
                if int((cum[mid] - cum[k]).max()) <= GTILES * P:
                    lo = mid
                else:
                    hi = mid - 1
            k2 = lo
            G = int((cum[k2] - cum[k]).max())
            gl = max(1, (G + P - 1) // P)
            ccum = cum[k:k2 + 1] - cum[k]
            tiles = []
            for j in range(gl):
                s0, s1 = j * P, (j + 1) * P - 1
                w0, w1 = Bc, -1
                for core in range(W):
                    n = int(ccum[-1, core])
                    if s0 >= n:
                        continue
                    e = min(s1, n - 1)
                    kf = int(np.searchsorted(ccum[:, core], s0, "right")) - 1
                    kl = int(np.searchsorted(ccum[:, core], e, "right")) - 1
                    w0 = min(w0, k + kf)
                    w1 = max(w1, k + kl)
                if w1 < w0:
                    w0, w1 = k, k
                tiles.append((w0, w1))
            raw.append((c, k, k2, gl, tiles))
            k = k2

    raw.reverse()   # short-batch (high k) groups first; low-k groups last

    # taper: split the final group into [gl-2, 1, 1] so the last gather's
    # DMA drain is short
    c, k0, k1, gl, tiles = raw[-1]
    pieces = [(c, k0, k1, gl, tiles, None)]
    if gl > 2:
        pieces = []
        s0 = 0
        for ngl in (gl - 2, 1, 1):
            pieces.append((c, k0, k1, ngl, tiles[s0:s0 + ngl],
                           (s0 * P, (s0 + ngl) * P)))
            s0 += ngl
    raw = [(cc, a, b, g, t, None) for (cc, a, b, g, t) in raw[:-1]] + pieces

    groups = []
    Tstart = 0
    col_off = 0
    for (c, k0, k1, gl, tiles, sub) in raw:
        g = dict(c=c, k0=k0, k1=k1, gl=gl, coff=col_off, Tstart=Tstart,
                 tiles=tiles)
        if sub is not None:
            g["sub"] = sub
        groups.append(g)
        col_off += gl * P // 16
        Tstart += gl

    T = Tstart
    widths = [w1 - w0 + 1 for g in groups for (w0, w1) in g["tiles"]]
    moff = np.zeros(T + 1, np.int64)
    moff[1:] = np.cumsum(widths)
    Wtot = int(moff[-1])

    nbank = (Bc + BANKC - 1) // BANKC
    flat = [(w0, w1) for g in groups for (w0, w1) in g["tiles"]]
    last_tile = {}
    for jg, (w0, w1) in enumerate(flat):
        for b in range(w0 // BANKC, w1 // BANKC + 1):
            last_tile[b] = jg
    parts = []  # per global tile: list of (bank, col0, col1, mask_local_off, stop)
    for jg, (w0, w1) in enumerate(flat):
        pj = []
        for b in range(w0 // BANKC, w1 // BANKC + 1):
            kb0 = max(w0, b * BANKC)
            kb1 = min(w1, b * BANKC + BANKC - 1)
            pj.append((b, kb0 - b * BANKC, kb1 - b * BANKC + 1,
                       kb0 - w0, jg == last_tile[b]))
        parts.append(pj)

    return dict(Bc=Bc, NCH=NCH, groups=groups, T=T, moff=moff, Wtot=Wtot,
                nbank=nbank, parts=parts, idx_cols=col_off)


def _trace_nc(st, V, DP):
    """Build + compile the SPMD Bacc program; DP = projected dim (128)."""
    Bc, Wtot = st["Bc"], st["Wtot"]
    moff, parts = st["moff"], st["parts"]
    nbank = st["nbank"]
    groups = st["groups"]
    idx_cols = st["idx_cols"]
    f32 = mybir.dt.float32
    bf16 = mybir.dt.bfloat16
    assert DP == P

    nc = bacc.Bacc(
        get_trn_type() or "TRN2",
        target_bir_lowering=False,
        debug=False,
        num_devices=NCORES,
        num_swdge_queues=NQ,
        dynamic_dma_scratch_size=32768,
    )
    ngroups = len(groups)
    t1_d = nc.dram_tensor("t1", [V, DP], bf16, kind="ExternalInput")
    idx_d = nc.dram_tensor("idx", [P, idx_cols], mybir.dt.int16,
                           kind="ExternalInput")
    mask_d = nc.dram_tensor("mask", [P, Wtot], bf16, kind="ExternalInput")
    b1_d = nc.dram_tensor("b1c", [P, 1], f32, kind="ExternalInput")
    w2t_d = nc.dram_tensor("w2t", [P, 1], bf16, kind="ExternalInput")
    b2_d = nc.dram_tensor("b2c", [1, 1], f32, kind="ExternalInput")
    y_d = nc.dram_tensor("y", [1, Bc], f32, kind="ExternalOutput")

    with tile.TileContext(nc) as tc, ExitStack() as ctx:
        consts = ctx.enter_context(tc.tile_pool(name="consts", bufs=1))
        gpool = ctx.enter_context(tc.tile_pool(name="gather", bufs=12))
        psum = ctx.enter_context(tc.tile_pool(name="psum", bufs=1, space="PSUM"))
        sb = ctx.enter_context(tc.tile_pool(name="sb", bufs=1))

        # Interleave idx chunks and mask slices on ONE queue: in-order
        # completion gives the first gather its idx fast while mask slices
        # land just ahead of the matmuls that need them.
        idx_sb = consts.tile([P, idx_cols], mybir.dt.int16)
        mask_sb = consts.tile([P, Wtot], bf16)
        bounds = [g["coff"] for g in groups] + [idx_cols]
        chunks = [(bounds[0], bounds[1])]
        i = 1
        while i < len(groups):
            j = min(i + 4, len(groups))
            chunks.append((bounds[i], bounds[j]))
            i = j
        nmsk = 16
        msk = [(Wtot * i // nmsk, Wtot * (i + 1) // nmsk) for i in range(nmsk)]
        mi = 0
        for ci, (lo, hi) in enumerate(chunks):
            if hi > lo:
                eng = nc.scalar if ci < 2 else nc.sync
                eng.dma_start(out=idx_sb[:, lo:hi],
                              in_=idx_d.ap()[:, lo:hi])
            if mi < nmsk:
                mlo, mhi = msk[mi]
                if mhi > mlo:
                    nc.sync.dma_start(out=mask_sb[:, mlo:mhi],
                                      in_=mask_d.ap()[:, mlo:mhi])
                mi += 1
        while mi < nmsk:
            mlo, mhi = msk[mi]
            if mhi > mlo:
                nc.sync.dma_start(out=mask_sb[:, mlo:mhi],
                                  in_=mask_d.ap()[:, mlo:mhi])
            mi += 1
        b1_sb = consts.tile([P, 1], f32)
        nc.sync.dma_start(out=b1_sb[:], in_=b1_d.ap())
        w2t_sb = consts.tile([P, 1], bf16)
        nc.sync.dma_start(out=w2t_sb[:], in_=w2t_d.ap())
        b2_sb = consts.tile([1, 1], f32)
        nc.sync.dma_start(out=b2_sb[:], in_=b2_d.ap())

        # Preload the sigmoid activation table off the critical tail.
        sig_warm = sb.tile([1, 1], f32)
        nc.scalar.activation(
            sig_warm[:], b2_sb[0:1, 0:1],
            mybir.ActivationFunctionType.Sigmoid,
        )

        # rep_ps[b] accumulates (W1 @ rep).T : [128 h, BANKC batches]
        rep_ps = [psum.tile([P, BANKC], f32, tag=f"rep{b}", name=f"rep{b}")
                  for b in range(nbank)]
        # Open each PSUM accumulation group with a full-bank zeroing matmul
        # (K=1, bf16) so every staircase matmul is a pure accumulate.
        zrow = consts.tile([1, BANKC], mybir.dt.bfloat16)
        nc.vector.memset(zrow, 0)
        for b in range(nbank):
            nc.tensor.matmul(
                rep_ps[b][:], zrow[0:1, 0:P], zrow[0:1, :],
                start=True, stop=False,
            )

        for gi, g in enumerate(groups):
            c, gl, coff, Tstart = g["c"], g["gl"], g["coff"], g["Tstart"]
            rows = min(CHUNK, V - c * CHUNK)
            chunk_ap = t1_d.ap()[c * CHUNK: c * CHUNK + rows, :]
            gt = gpool.tile([P, GTILES, DP], bf16, tag="gt")
            nc.gpsimd.dma_gather(
                gt[:, :gl, :],
                chunk_ap,
                idx_sb[:, coff: coff + gl * P // 16],
                gl * P,
                gl * P,
                DP,
                single_packet=False,
                queue_num=gi % NQ,
            )
            for jl in range(gl):
                jg = Tstart + jl
                mo = int(moff[jg])
                lhsT = gt[:, jl, :]
                for (b, c0, c1, ml, sp_flag) in parts[jg]:
                    nc.tensor.matmul(
                        rep_ps[b][:, c0:c1],
                        lhsT,
                        mask_sb[:, mo + ml: mo + ml + (c1 - c0)],
                        start=False,
                        stop=sp_flag,
                    )

        # ---- tail: h = relu(rep_proj + b1) on DVE (bf16 out);
        #      y = sigmoid(W2 @ h + b2) ----
        h_sb = sb.tile([P, Bc], bf16)
        for b in range(nbank):
            nc.vector.tensor_scalar(
                h_sb[:, b * BANKC:(b + 1) * BANKC],
                rep_ps[b][:],
                b1_sb[:, 0:1],
                0.0,
                mybir.AluOpType.add,
                mybir.AluOpType.max,
            )
        l_ps = [psum.tile([1, BANKC], f32, tag=f"lps{b}", name=f"lps{b}")
                for b in range(nbank)]
        y_sb = sb.tile([1, Bc], f32)
        for b in range(nbank):
            nc.tensor.matmul(
                l_ps[b][:],
                w2t_sb[:],
                h_sb[:, b * BANKC:(b + 1) * BANKC],
                start=True, stop=True,
            )
            nc.scalar.activation(
                y_sb[:, b * BANKC:(b + 1) * BANKC],
                l_ps[b][:],
                mybir.ActivationFunctionType.Sigmoid,
                bias=b2_sb[0:1, 0:1],
            )
            nc.sync.dma_start(out=y_d.ap()[:, b * BANKC:(b + 1) * BANKC],
                              in_=y_sb[:, b * BANKC:(b + 1) * BANKC])

    nc.compile()
    return nc


def _local_search(perm, nb, iters=4):
    """Swap batches between adjacent rows to tighten per-row max counts."""
    Bc, W = perm.shape
    for _ in range(iters):
        changed = False
        for parity in (0, 1):
            ks = np.arange(parity, Bc - 1, 2)
            for a in range(W):
                for b in range(W):
                    Qa = nb[perm[ks]]
                    Qb = nb[perm[ks + 1]]
                    old = Qa.max(1).sum(1) + Qb.max(1).sum(1)
                    Qa2 = Qa.copy()
                    Qb2 = Qb.copy()
                    Qa2[:, a] = nb[perm[ks + 1, b]]
                    Qb2[:, b] = nb[perm[ks, a]]
                    new = Qa2.max(1).sum(1) + Qb2.max(1).sum(1)
                    win = new < old
                    if win.any():
                        kw = ks[win]
                        tmp = perm[kw, a].copy()
                        perm[kw, a] = perm[kw + 1, b]
                        perm[kw + 1, b] = tmp
                        changed = True
        if not changed:
            break
    return perm


def _prepare(x, lengths, emb_table, W1, b1, W2, b2):
    """Host-side sharding: weight fusion + canonical structure + arrays."""
    x = np.asarray(x)
    lengths = np.asarray(lengths).astype(np.int64)
    B, L = x.shape
    V, D = emb_table.shape
    Bc = B // NCORES
    NCH = (V + CHUNK - 1) // CHUNK

    # weight fusion: masked-mean commutes with W1
    W1f = np.asarray(W1, np.float32)
    t1 = np.ascontiguousarray(
        np.asarray(emb_table, np.float32) @ W1f.T).astype(bfloat16)  # [V, 128]
    DP = t1.shape[1]

    lpos = np.arange(L, dtype=np.int64)
    chunk_of = (x >> 15).astype(np.int64)     # CHUNK == 1 << 15
    valid_all = lpos[None, :] < lengths[:, None]
    nb = np.zeros((B, NCH), np.int64)         # per-batch per-chunk counts
    for c in range(NCH):
        nb[:, c] = ((chunk_of == c) & valid_all).sum(axis=1)

    # Sort by length desc, cluster chunk compositions within near-equal
    # length bands, then local-search swaps between adjacent rows.
    order = np.argsort(-lengths, kind="stable")
    ln_sorted = lengths[order]
    BAND = 8
    i = 0
    while i < B:
        j = i
        while j < B and ln_sorted[i] - ln_sorted[j] < BAND:
            j += 1
        band = order[i:j]
        v = nb[band]
        order[i:j] = band[np.lexsort(
            (v[:, 3 % NCH], v[:, 2 % NCH], v[:, 1 % NCH], v[:, 0]))]
        i = j
    perm = order.reshape(Bc, NCORES)          # [k, core] -> original batch idx
    perm = _local_search(perm, nb)
    plen = lengths[perm]                      # [k, core] actual lengths

    counts = nb[perm]                         # [Bc, NCORES, NCH]
    st = _build_structure(counts, V)
    groups, moff = st["groups"], st["moff"]
    Wtot, idx_cols = st["Wtot"], st["idx_cols"]

    inv_len = (1.0 / plen.astype(np.float64)).astype(np.float32)

    idx_cores = []
    mask_cores = []
    for core in range(NCORES):
        xl = x[perm[:, core]]
        validc = lpos[None, :] < plen[:, core][:, None]
        idx16 = np.zeros((P, idx_cols), np.int16)
        mask_host = np.zeros((P, Wtot), np.float32)
        for c in range(NCH):
            sel_mask = validc & (chunk_of[perm[:, core]] == c)
            sel = np.nonzero(sel_mask.ravel())[0]
            k_sel = sel // L                       # sorted ascending
            ids = (xl.ravel()[sel] & (CHUNK - 1)).astype(np.int16)
            cnt = np.bincount(k_sel, minlength=Bc)
            ccum = np.zeros(Bc + 1, np.int64)
            ccum[1:] = np.cumsum(cnt)
            for gi, g in enumerate(groups):
                if g["c"] != c:
                    continue
                k0, k1, gl, coff, Tstart = (g["k0"], g["k1"], g["gl"],
                                            g["coff"], g["Tstart"])
                a, bnd = int(ccum[k0]), int(ccum[k1])
                if "sub" in g:                     # tapered sub-group
                    lo, hi = g["sub"]
                    a, bnd = a + lo, min(bnd, a + hi)
                n = max(0, bnd - a)
                gslots = gl * P
                seg = np.zeros(gslots, np.int16)
                seg[:n] = ids[a:bnd]
                # packed slot s -> batch row k_sel[a+s], tile j = s//P
                if n:
                    s = np.arange(n)
                    kk = k_sel[a:bnd]             # global row index
                    j_in_g = s // P
                    jg = Tstart + j_in_g
                    w0 = np.array([g["tiles"][j][0] for j in range(gl)])
                    col = moff[jg] + (kk - w0[j_in_g])
                    mask_host[s % P, col] = inv_len[kk, core]
                wrap = seg.reshape(-1, 16).T
                for st8 in range(P // 16):
                    idx16[16 * st8:16 * st8 + 16,
                          coff: coff + gslots // 16] = wrap
        idx_cores.append(idx16)
        mask_cores.append(mask_host.astype(bfloat16))

    b1c = np.asarray(b1, np.float32).reshape(P, 1)
    w2t = np.ascontiguousarray(
        np.asarray(W2, np.float32).reshape(1, P).T).astype(bfloat16)
    b2c = np.asarray(b2, np.float32).reshape(1, 1)

    in_maps = []
    for core in range(NCORES):
        in_maps.append({
            "t1": t1,
            "idx": idx_cores[core],
            "mask": mask_cores[core],
            "b1c": b1c,
            "w2t": w2t,
            "b2c": b2c,
        })
    return st, perm, in_maps, (V, DP)


def kernel(x, lengths, emb_table, W1, b1, W2, b2):
    global LAST_RESULT
    st, perm, in_maps, (V, DP) = _prepare(x, lengths, emb_table, W1, b1, W2, b2)

    key = (st["T"], st["Wtot"], V, DP, st["Bc"])
    nc = _NC_CACHE.get(key)
    if nc is None:
        nc = _trace_nc(st, V, DP)
        _NC_CACHE[key] = nc

    trace = bool(int(os.environ.get("KERNEL_TRACE", "0")))
    res = run_bass_kernel_spmd(nc, in_maps, core_ids=list(range(NCORES)),
                               trace=trace)
    LAST_RESULT = res

    B = perm.size
    out = np.zeros(B, np.float32)
    for c in range(NCORES):
        out[perm[:, c]] = res.results[c]["y"][0]
    return out
